# revision 3
# baseline (speedup 1.0000x reference)
"""GCN layer on 8 Trainium2 cores. v7 over v6 (streamed S_val):

- pads carry a valid dummy index (0) instead of -1, so num_idxs_reg is the
  compile-time CAP: no per-call reg_loads, no ucode trailing-trim scan, no
  first-use ebuf memsets (every slot is gather-written; sval=0 nullifies
  pad slots in the matmul).
- rows are LPT bin-packed into tiles (output rows are permuted, host
  unscrambles), equalizing per-tile edge counts; buckets are exact
  positional quartiles of each tile's col-sorted edges, clipped to the
  overlapping int16 windows -> qb drops to 8 (CAP 1024), cutting the
  serial dma_gather descgen scan/emit work ~11%.
"""

import sys

sys.path.insert(0, "/opt/trn_rl_repo")

import numpy as np

N_NODES = 100000
D = 128
LEAKY_SLOPE = 0.5
N_CORES = 8
ROWS_PER_CORE = 12500
TILE_ROWS = 128
TILES = 98  # 98*128 = 12544 >= 12500
PAD_ROWS = TILES * TILE_ROWS
NB = 4  # column buckets == SWDGE queues
BASES = (0, 22500, 45000, 67500)  # overlapping int16-addressable windows
WIN = 32768
OUT_FLUSH_TILES = 7
EBUF_BUFS = 8

_BUILD_CACHE = {}


def _build_bass(qb: int, repeat: int = 1, nqueues: int = NB,
                do_gather: bool = True, do_compute: bool = True):
    """qb = 128-edge chunks per (tile, bucket) segment."""
    import contextlib

    import concourse.bacc as bacc
    import concourse.mybir as mybir
    import concourse.tile as tile

    f32 = mybir.dt.float32
    i16 = mybir.dt.int16
    bf16 = mybir.dt.bfloat16

    CAP = qb * 128  # capacity per (tile, bucket)
    NCHUNK = NB * qb  # chunks per tile
    IDX_COLS = NB * CAP // 16

    nc = bacc.Bacc("TRN2", target_bir_lowering=False, debug=False,
                   num_devices=N_CORES, num_swdge_queues=nqueues)

    emb = nc.dram_tensor("embeds", [N_NODES, D], bf16, kind="ExternalInput")
    svals = nc.dram_tensor("svals", [TILES, 128, NCHUNK * TILE_ROWS], bf16,
                           kind="ExternalInput")
    meta16 = nc.dram_tensor("meta16", [TILES, 128, IDX_COLS], i16,
                            kind="ExternalInput")
    out = nc.dram_tensor("out", [D, PAD_ROWS], f32, kind="ExternalOutput")

    with tile.TileContext(nc) as tc:
        with (
            tc.tile_pool(name="meta", bufs=4) as meta_pool,
            tc.tile_pool(name="ebuf", bufs=EBUF_BUFS) as ebuf_pool,
            tc.tile_pool(name="sval", bufs=4) as sval_pool,
            tc.tile_pool(name="evac", bufs=4) as evac_pool,
            tc.tile_pool(name="acc", bufs=2) as acc_pool,
            tc.tile_pool(name="psum", bufs=6, space="PSUM") as psum_pool,
        ):
            out_cols = OUT_FLUSH_TILES * TILE_ROWS  # 896
            if repeat > 1:
                loop_cm = tc.For_i(
                    0, repeat, 1,
                    hint_engines=(
                        mybir.EngineType.PE,
                        mybir.EngineType.DVE,
                        mybir.EngineType.Pool,
                        mybir.EngineType.SP,
                        mybir.EngineType.Activation,
                    ),
                )
            else:
                loop_cm = contextlib.nullcontext()
            with loop_cm:
                LOOKAHEAD = 6
                metas = {}
                ebufs = {}
                accs = {}

                def issue_meta(t):
                    if t >= TILES:
                        return
                    m16_t = meta_pool.tile([128, IDX_COLS], i16, tag="m16")
                    nc.sync.dma_start(out=m16_t[:], in_=meta16.ap()[t])
                    sv_t = sval_pool.tile([128, NCHUNK * TILE_ROWS], bf16,
                                          tag="sv")
                    nc.sync.dma_start(out=sv_t[:], in_=svals.ap()[t])
                    metas[t] = (m16_t, sv_t)

                def issue_gather(t):
                    if t >= TILES or not do_gather:
                        return
                    m16_t, sv_t = metas[t]
                    e_t = ebuf_pool.tile([128, NCHUNK, D], bf16, tag="ebuf")
                    for j in range(NB):
                        hi = min(BASES[j] + WIN, N_NODES)
                        nc.gpsimd.dma_gather(
                            out_ap=e_t[:, j * qb:(j + 1) * qb, :],
                            in_ap=emb.ap()[BASES[j]:hi, :],
                            idxs_ap=m16_t[:, j * (CAP // 16):
                                          (j + 1) * (CAP // 16)],
                            num_idxs=CAP,
                            num_idxs_reg=CAP,
                            elem_size=D,
                            elem_step=D,
                            single_packet=False,
                            queue_num=j % nqueues,
                        )
                    ebufs[t] = e_t

                def issue_compute(t):
                    m16_t, sv_t = metas.pop(t)
                    if do_gather:
                        e_t = ebufs.pop(t)
                    else:
                        e_t = ebuf_pool.tile([128, NCHUNK, D], bf16,
                                             tag="ebuf")
                        nc.vector.memset(e_t[:, 0, :], 0.0)
                    if t % OUT_FLUSH_TILES == 0:
                        acc_new = acc_pool.tile([128, out_cols], f32,
                                                tag="acc")
                        accs[0] = acc_new
                    acc_t = accs[0]
                    ps = psum_pool.tile([128, TILE_ROWS], f32, tag="ps")
                    for s in (range(NCHUNK) if do_compute else range(1)):
                        nc.tensor.matmul(
                            ps[:],
                            lhsT=e_t[:, s, :],
                            rhs=sv_t[:, s * TILE_ROWS:(s + 1) * TILE_ROWS],
                            start=(s == 0),
                            stop=(s == NCHUNK - 1) or not do_compute,
                        )
                    col0 = (t % OUT_FLUSH_TILES) * TILE_ROWS
                    half_t = evac_pool.tile([128, TILE_ROWS], f32,
                                            tag="half")
                    nc.scalar.mul(half_t[:], ps[:], LEAKY_SLOPE)
                    nc.vector.tensor_tensor(
                        out=acc_t[:, col0:col0 + TILE_ROWS],
                        in0=ps[:],
                        in1=half_t[:],
                        op=mybir.AluOpType.max,
                    )
                    if t % OUT_FLUSH_TILES == OUT_FLUSH_TILES - 1:
                        c0 = (t - (OUT_FLUSH_TILES - 1)) * TILE_ROWS
                        nc.sync.dma_start(
                            out=out.ap()[:, c0:c0 + out_cols],
                            in_=acc_t[:],
                        )

                for t in range(LOOKAHEAD):
                    issue_meta(t)
                    issue_gather(t)
                for t in range(TILES):
                    issue_meta(t + LOOKAHEAD)
                    issue_gather(t + LOOKAHEAD)
                    issue_compute(t)
    nc.compile()
    return nc


def _pack_rows(core, r_in_core):
    """Pack each core's rows into TILES tiles of <=128 rows so per-tile
    edge counts are near-equal (serpentine deal of count-sorted rows plus
    a swap refinement). Returns (tile_of, rl_of) [N_CORES, ROWS]."""
    tile_of = np.empty((N_CORES, ROWS_PER_CORE), np.int64)
    rl_of = np.empty((N_CORES, ROWS_PER_CORE), np.int64)
    for c in range(N_CORES):
        cnts_r = np.bincount(r_in_core[core == c], minlength=ROWS_PER_CORE)
        order_r = np.argsort(-cnts_r, kind="stable")
        tl = np.empty(ROWS_PER_CORE, np.int64)
        sums = np.zeros(TILES, np.int64)
        ti, step = 0, 1
        for r in order_r:
            tl[r] = ti
            sums[ti] += cnts_r[r]
            nxt = ti + step
            if nxt < 0 or nxt >= TILES:
                step = -step
            else:
                ti = nxt
        # swap refinement: trade one row between heaviest/lightest tiles
        for _ in range(200):
            h = int(np.argmax(sums))
            l = int(np.argmin(sums))
            gap = sums[h] - sums[l]
            if gap <= 2:
                break
            rows_h = np.where(tl == h)[0]
            rows_l = np.where(tl == l)[0]
            want = gap // 2
            dh = cnts_r[rows_h]
            dl = cnts_r[rows_l]
            diff = dh[:, None] - dl[None, :]
            good = np.abs(diff - want)
            ih, il = np.unravel_index(np.argmin(good), good.shape)
            if diff[ih, il] <= 0:
                break
            a, b = rows_h[ih], rows_l[il]
            tl[a], tl[b] = l, h
            sums[h] -= diff[ih, il]
            sums[l] += diff[ih, il]
        tile_of[c] = tl
        # rl = slot within tile, in row order
        ordr = np.lexsort((np.arange(ROWS_PER_CORE), tl))
        pos = np.empty(ROWS_PER_CORE, np.int64)
        fills = np.zeros(TILES, np.int64)
        for r in ordr:
            pos[r] = fills[tl[r]]
            fills[tl[r]] += 1
        assert fills.max() <= TILE_ROWS
        rl_of[c] = pos
    return tile_of, rl_of


def _prep_inputs(edge_index, edge_vals, embeds):
    import ml_dtypes

    bf = ml_dtypes.bfloat16
    row = np.asarray(edge_index[0], dtype=np.int64)
    col = np.asarray(edge_index[1], dtype=np.int64)
    val = np.asarray(edge_vals, dtype=np.float32)
    embeds = np.ascontiguousarray(
        np.asarray(embeds, dtype=np.float32).astype(bf)
    )

    core = row // ROWS_PER_CORE
    r_in_core = row - core * ROWS_PER_CORE

    tile_of, rl_of = _pack_rows(core, r_in_core)
    t_idx = tile_of[core, r_in_core]
    rl = rl_of[core, r_in_core]

    tile_id = core * TILES + t_idx
    ntiles = N_CORES * TILES

    # sort by (tile, col); buckets are positional quartiles of each tile's
    # col-sorted edges, split points clipped into the window-feasible range
    order = np.lexsort((col, tile_id))
    tsort = tile_id[order]
    csort = col[order]
    rl2 = rl[order]
    val2 = val[order]
    tcounts = np.bincount(tile_id, minlength=ntiles)
    tstarts = np.cumsum(tcounts) - tcounts

    j_sorted = np.empty(row.size, dtype=np.int64)
    for g in range(ntiles):
        s0, n = tstarts[g], tcounts[g]
        seg = csort[s0:s0 + n]
        prev = 0
        for k in range(1, NB):
            lo = np.searchsorted(seg, BASES[k])          # first col >= base_k
            hi = np.searchsorted(seg, BASES[k - 1] + WIN)  # first col >= end
            p = min(max((k * n) // NB, lo), hi)
            p = max(p, prev)
            j_sorted[s0 + prev:s0 + p] = k - 1
            prev = p
        j_sorted[s0 + prev:s0 + n] = NB - 1

    bases = np.asarray(BASES, dtype=np.int64)
    off = csort - bases[j_sorted]
    assert off.min() >= 0 and off.max() < WIN

    seg2 = tsort * NB + j_sorted  # already sorted (buckets are pos ranges)
    nseg = ntiles * NB
    counts = np.bincount(seg2, minlength=nseg)
    qb = max(2, int(-(-counts.max() // 128)))
    cap = qb * 128

    starts = np.cumsum(counts) - counts
    pos = np.arange(row.size, dtype=np.int64) - starts[seg2]
    slots = seg2 * cap + pos

    n_slots = nseg * cap
    # pads hold a valid dummy idx (0): gathered then nulled by sval=0
    idx16 = np.zeros(n_slots, dtype=np.int16)
    idx16[slots] = off.astype(np.int16)

    # streamed S_val: svals[c, t, e, s*128 + r] = val * (rl == r)
    NCHUNK = NB * qb
    sl_seg = slots // cap
    sl_t = sl_seg // NB
    sl_j = sl_seg % NB
    sl_q = (slots % cap) // 128
    sl_e = slots % 128
    sl_s = sl_j * qb + sl_q
    sv = np.zeros((ntiles, 128, NCHUNK * TILE_ROWS), dtype=bf)
    sv[sl_t, sl_e, sl_s * TILE_ROWS + rl2] = val2.astype(bf)
    svals = sv.reshape(N_CORES, TILES, 128, NCHUNK * TILE_ROWS)

    # meta16: per call j, idx i = q*128 + p lives at (p%16, q*8 + p//16)
    a = idx16.reshape(N_CORES, TILES, NB, qb, 8, 16)  # p = p_hi*16 + p_lo
    a = a.transpose(0, 1, 5, 2, 3, 4)  # [c, t, p_lo, j, q, p_hi]
    a = a.reshape(N_CORES, TILES, 16, NB * qb * 8)
    meta16 = np.ascontiguousarray(np.tile(a, (1, 1, 8, 1)))

    # output unscramble: core c's row r lives at column tile*128 + rl
    colpos = tile_of * TILE_ROWS + rl_of  # [N_CORES, ROWS_PER_CORE]

    return embeds, svals, meta16, colpos, qb


def _make_in_maps(embeds_np, svals, meta16, colpos):
    return [
        {"embeds": embeds_np, "svals": svals[c], "meta16": meta16[c]}
        for c in range(N_CORES)
    ]


def kernel(edge_index, edge_vals, embeds):
    from concourse.bass_utils import run_bass_kernel_spmd

    embeds_np, svals, meta16, colpos, qb = _prep_inputs(
        edge_index, edge_vals, embeds
    )

    if qb not in _BUILD_CACHE:
        _BUILD_CACHE[qb] = _build_bass(qb)
    nc = _BUILD_CACHE[qb]

    in_maps = _make_in_maps(embeds_np, svals, meta16, colpos)
    res = run_bass_kernel_spmd(nc, in_maps, core_ids=list(range(N_CORES)))

    out_full = np.empty((N_NODES, D), dtype=np.float32)
    for c in range(N_CORES):
        oc = res.results[c]["out"]  # [D, PAD_ROWS]
        out_full[c * ROWS_PER_CORE:(c + 1) * ROWS_PER_CORE] = \
            oc[:, colpos[c]].T
    return out_full
